# revision 55
# baseline (speedup 1.0000x reference)
"""Single-head attention (shared-input QKV projections) on 8 Trainium2 cores.

Reference computation (per batch b):
    q = x[b] @ Wq; k = x[b] @ Wk; v = x[b] @ Wv        # [S, 64]
    out[b] = softmax(q @ k.T / 8) @ v                  # [S, 64]
with B=4, S=4096, D=256, OUT=64.

Sharding: data-parallel over batch (4 batches x 2 cores) with
sequence-parallel query halves; per-core query offset handled by host-side
row rotation of x[b] (attention is permutation-invariant over key rows).

Per-core kernel. The softmax exp stream (8.4M elements/core) would be a
single-engine wall on ScalarE, so whole score tiles alternate between
BOTH ScalarE (true `exp` activation) and VectorE (Schraudolph bit-trick
exp: ONE tensor_scalar add whose int16 output bit pattern IS ~exp in
bf16; ~3% sawtooth error that the softmax normalization largely
cancels -- end-to-end rel err ~6e-3 vs the 2e-2 gate):
  1. DMA x^T (d-major) and W into SBUF, all bf16 (halves HBM traffic;
     the PE rounds fp32 inputs to ~tf32 anyway). Host pre-scales Wk by
     A_K = SCALE*2^7/ln2 so raw scores land in the bf16-Schraudolph
     integer domain; the ACT path undoes this with scale=ln2/2^7.
  2. Projections: Q^T/K^T bf16 (fast-weight-load halves LDWEIGHTS),
     duplicated across both 64-partition halves for 2-way row-packed
     K=64 score matmuls. V is produced directly in [key, out] chunk
     layout by swapping matmul roles (stationary = x chunk, moving =
     Wv), with a ones column per chunk so attn @ V_aug also yields the
     softmax denominator for free.
  3. Scores computed transposed (S^T[k, q]): per (q-block, k-chunk-pair)
     2 row-packed matmuls -> PSUM [128, 2W] (3-deep tile ring, software
     pipelined 2 pairs deep so exp latency never stalls the PE); exp on
     ACT or DVE (by pair parity) -> bf16 SBUF; 2 accumulating
     attn @ V_aug matmuls into PSUM [65, W].
  4. Epilogue: copy PSUM [65, W] numerator+denominator rows to SBUF,
     DMA out. Normalization (divide by the ones-row) and the
     [o,q]->[q,o] transpose happen on the host.
"""

import numpy as np

import concourse.mybir as mybir
import concourse.tile as tile
from concourse import bacc

P = 128
D = 256
OUT = 64
SCALE = 0.125
LN2 = float(np.log(2.0))
A_K = SCALE * (1 << 7) / LN2        # folded into Wk on host (bf16 domain)
ACT_SCALE = LN2 / (1 << 7)          # undoes A_K inside the exp activation
C_MAGIC = 6.0                       # Schraudolph centering (sim-tuned)
B_MAGIC = 127.0 * (1 << 7) - C_MAGIC
F32 = mybir.dt.float32
F32R = mybir.dt.float32r
BF16 = mybir.dt.bfloat16
I16 = mybir.dt.int16

B_FULL, S_FULL = 4, 4096
N_CORES = 8
Z_ACT = 544  # columns of each [128,1024] score tile that ACT exps; DVE rest


def build_nc(S: int, QH: int, QB_W: int = 512, loop_n: int | None = None,
             timing_mode: bool = False, variant: str = "full", bodies: int = 1):
    """Build the per-core SPMD program.

    S: sequence length (key/value rows) held by this core.
    QH: number of query rows this core computes (first QH rows of x).
    QB_W: query block width (free dim of the score matmuls).
    loop_n: if set, run the whole body loop_n times on device (for timing).
    timing_mode: shrink the xt input to 512 cols (replicated on device) so
        host->device transfer noise doesn't swamp loop-delta timing.
    """
    assert S % 512 == 0 and QH % QB_W == 0 and QB_W % P == 0
    nc = bacc.Bacc()
    xt_cols = 512 if timing_mode else S
    xt_in = nc.declare_dram_parameter("xt", [2, P, xt_cols], BF16, isOutput=False)
    w_in = nc.declare_dram_parameter("w", [3, D, P], BF16, isOutput=False)
    num_d = nc.declare_dram_parameter("num", [OUT + 1, QH], F32, isOutput=True)

    with tile.TileContext(nc) as tc:
        with (
            tc.tile_pool(name="const", bufs=1) as constp,
            tc.tile_pool(name="big", bufs=1) as bigp,
            tc.tile_pool(name="attnp", bufs=8) as attnp,
            tc.tile_pool(name="epil", bufs=2) as epilp,
            tc.tile_pool(name="stps", bufs=3, space="PSUM") as stps,
            tc.tile_pool(name="pops", bufs=2, space="PSUM") as pops,
        ):
            miscps = stps  # proj/V scratch shares the score ring pool
            # Weights split across the two HWDGE queues (SP carries q/k,
            # ACT carries v) so the first projection's deps land early.
            w_sb = constp.tile([P, 6 * P], BF16)
            for j in range(3):
                eng = nc.sync if j < 2 else nc.scalar
                for c in range(2):
                    eng.dma_start(
                        w_sb[:, (j * 2 + c) * P : (j * 2 + c + 1) * P],
                        w_in[j, c * P : (c + 1) * P, :],
                    )
            # V_aug lives across loop iterations: the V chunk copies only
            # touch columns 0:64 of each 65-stride group, so the ones
            # column (the softmax-denominator trick) is written once here.
            v_sb = bigp.tile([P, (S // P) * 65], BF16, name="v_sb")
            ones32 = constp.tile([P, S // P], F32)
            nc.vector.memset(ones32, 1.0)
            nc.vector.tensor_copy(
                v_sb.rearrange("p (k c) -> p k c", c=65)[:, :, 64], ones32
            )
            # Trigger the ACT exp table-set load (~2.7us) during the input
            # DMA ramp instead of stalling the first score tile's exp.
            warm = constp.tile([P, 1], BF16, name="warm")
            nc.scalar.activation(
                warm, w_sb[:, 0:1], mybir.ActivationFunctionType.Exp,
                scale=ACT_SCALE,
            )
            if loop_n is not None:
                loop_cm = tc.For_i(0, loop_n, 1)
                loop_cm.__enter__()
            for _ in range(bodies):
                _emit_body(nc, tc, xt_in, num_d, S, QH, QB_W, constp, bigp,
                           attnp, epilp, miscps, stps, pops, w_sb, v_sb,
                           timing_mode, variant)
            if loop_n is not None:
                loop_cm.__exit__(None, None, None)
    return nc


def _emit_body(nc, tc, xt_in, num_d, S, QH, QB_W, constp, bigp, attnp,
               epilp, miscps, stps, pops, w_sb, v_sb, timing_mode=False,
               variant="full"):
    do_dma = variant != "nodma"
    do_st = variant not in ("proj",)
    do_exp = do_st and variant not in ("scores",)
    do_av = do_exp and variant not in ("noav",)
    nk = S // P          # 128-row k chunks
    npair = nk // 2      # row-packed chunk pairs
    nqb = QH // QB_W     # q blocks
    qpb = min(512, QH)   # q-projection block width

    # x^T: chunk c at cols [c*S, (c+1)*S); DMA'd in 1024-col slices so
    # downstream projections wait only on the slices they read. Chunk 0
    # rides the SP HWDGE queue, chunk 1 the ACT queue.
    xt = bigp.tile([P, 2 * S], BF16)
    xb_w = min(512 if timing_mode else 1024, S)
    slices = [(lo, xb_w) for lo in range(0, S, xb_w)]
    if slices[0][1] > 512:
        slices = [(0, 512), (512, xb_w - 512)] + slices[1:]
    if do_dma:
        for lo, wdt in slices:
            for c in range(2):
                eng = nc.sync if c == 0 else nc.scalar
                src_lo = 0 if timing_mode else lo
                eng.dma_start(
                    xt[:, c * S + lo : c * S + lo + wdt],
                    xt_in[c, :, src_lo : src_lo + wdt],
                )
    else:
        nc.sync.dma_start(xt[:, 0:512], xt_in[0, :, 0:512])

    qt = bigp.tile([P, QH], BF16)
    kt = bigp.tile([P, S], BF16)

    def proj_block(dst, j, lo, width, eng):
        """dst[:, lo:lo+width] = (W_j^T x^T)[:, lo:lo+width] (d contracted)."""
        pp = miscps.tile([P, width], F32, name="pp", tag="st")
        for c in range(2):
            nc.tensor.matmul(
                pp,
                w_sb[:, (j * 2 + c) * P : (j * 2 + c + 1) * P],
                xt[:, c * S + lo : c * S + lo + width],
                start=(c == 0),
                stop=(c == 1),
            )
        eng(dst[:, lo : lo + width], pp)

    def v_group(g):
        """V rows [g*512,(g+1)*512) directly in [k, o] chunk layout:
        stationary = x chunk (d-major), moving = Wv half."""
        pv = miscps.tile([P, 4 * OUT], F32, name="pv", tag="st")
        for i, kc in enumerate(range(4 * g, 4 * g + 4)):
            for h in range(2):
                nc.tensor.matmul(
                    pv[:, i * OUT : (i + 1) * OUT],
                    xt[:, h * S + kc * P : h * S + (kc + 1) * P],
                    w_sb[:, (4 + h) * P : (4 + h) * P + OUT],
                    start=(h == 0),
                    stop=(h == 1),
                )
        dst = v_sb.rearrange("p (k c) -> p k c", c=65)[:, 4 * g : 4 * g + 4, 0:64]
        nc.vector.tensor_copy(dst, pv.rearrange("p (k c) -> p k c", c=OUT))

    po_tiles = {}

    def st_part(qb, t):
        """Score matmuls (S^T) for q block qb, k chunk pair t -> PSUM tile."""
        qs = qb * QB_W
        kca, kcb = 2 * t, 2 * t + 1
        st = stps.tile([P, 2 * QB_W], F32, name="st", tag="st")
        nc.tensor.matmul(
            st[:, 0:QB_W],
            kt[0:64, kca * P : (kca + 1) * P],
            qt[0:64, qs : qs + QB_W],
            start=True,
            stop=True,
        )
        nc.tensor.matmul(
            st[:, QB_W : 2 * QB_W],
            kt[64:128, kcb * P : (kcb + 1) * P],
            qt[64:128, qs : qs + QB_W],
            start=True,
            stop=True,
        )
        return st

    def exp_part(qb, t, st):
        """exp of a whole score pair-tile on ONE engine, alternating by
        (qb+t) parity: ACT runs the true exp activation, DVE the Schraudolph
        int16 bit-trick. Whole-tile instructions amortize the fixed
        per-instruction engine overhead (ACT: 352 cyc); the 3-deep score
        ring keeps the st-reuse WAR chain off the critical path."""
        at = attnp.tile([P, 2 * QB_W], BF16, name="at", tag="at")
        idx = (qb + t) % 16
        if idx % 2 == 0:
            nc.scalar.activation(
                at, st, mybir.ActivationFunctionType.Exp, scale=ACT_SCALE
            )
        else:
            nc.vector.tensor_scalar(
                at.bitcast(I16), st, B_MAGIC, None, mybir.AluOpType.add
            )
        return at

    def av_part(qb, t, at):
        """attn@V_aug accumulation for q block qb, k chunk pair t."""
        kca, kcb = 2 * t, 2 * t + 1
        po = po_tiles[qb]
        nc.tensor.matmul(
            po,
            v_sb[:, kca * 65 : (kca + 1) * 65],
            at[:, 0:QB_W],
            start=(t == 0),
            stop=False,
        )
        nc.tensor.matmul(
            po,
            v_sb[:, kcb * 65 : (kcb + 1) * 65],
            at[:, QB_W : 2 * QB_W],
            start=False,
            stop=(t == npair - 1),
        )

    def exp_av_part(qb, t, st):
        if not do_exp:
            return
        at = exp_part(qb, t, st)
        if do_av:
            av_part(qb, t, at)

    def epilogue(qb):
        qs = qb * QB_W
        o_sb = epilp.tile([OUT + 1, QB_W], F32, name="o_sb", tag="o_sb")
        if do_av:
            po = po_tiles.pop(qb)
            if qb % 2 == 0:
                nc.scalar.copy(o_sb, po)
            else:
                nc.vector.tensor_copy(o_sb, po)
        else:
            nc.vector.memset(o_sb, 0.0)
        nc.sync.dma_start(num_d[:, qs : qs + QB_W], o_sb)

    # --- emission ---
    # Phase 1: interleave k-chunk production (kt blocks, V groups) with the
    # first two q blocks' score/exp/AV streams, software-pipelined one combo
    # deep so the exp engines never wait on a fresh S^T + semaphore.
    lead = min(2, nqb)
    for lo in range(0, lead * QB_W, qpb):
        proj_block(qt, 0, lo, min(qpb, QH - lo),
                   lambda d, s: nc.vector.tensor_copy(d, s))
    if do_av:
        for qb in range(lead):
            po_tiles[qb] = pops.tile([OUT + 1, QB_W], F32, name="po", tag="po")

    # Software pipeline two score tiles deep (the stps pool holds 3): the
    # exp->attn@V chain of pair p only has to beat TWO later score pairs'
    # PE time, keeping the PE free of exp-latency stalls.
    pend = []

    def push_pend(item):
        pend.append(item)
        if len(pend) > 2:
            exp_av_part(*pend.pop(0))

    def flush_pend():
        while pend:
            exp_av_part(*pend.pop(0))

    for g in range(S // 512):  # 512 k rows per group = 4 chunks = 2 pairs
        proj_block(kt, 1, g * 512, 512,
                   lambda d, s: nc.scalar.copy(d, s))
        v_group(g)
        # Interleave the remaining q projections into phase 1 so the phase
        # boundary has no PE bubble and xt's last reader retires early
        # (lets the next iteration's input DMA start sooner).
        qrest = list(range(lead * QB_W, QH, qpb))
        if g >= 4 and (g - 4) < len(qrest):
            lo = qrest[g - 4]
            proj_block(qt, 0, lo, min(qpb, QH - lo),
                       lambda d, s: nc.vector.tensor_copy(d, s))
        if do_st:
            for t in (2 * g, 2 * g + 1):
                for qb in range(lead):
                    push_pend((qb, t, st_part(qb, t)))
    flush_pend()

    for qb in range(lead):
        epilogue(qb)

    # Phase 2: pure streaming q blocks, software-pipelined one pair deep.
    for qb in range(lead, nqb):
        if do_av:
            po_tiles[qb] = pops.tile([OUT + 1, QB_W], F32, name="po", tag="po")
        if do_st:
            for t in range(npair):
                push_pend((qb, t, st_part(qb, t)))
            flush_pend()
        epilogue(qb)


_compiled_nc = None
LAST_RESULT = None  # BassKernelResults of the most recent kernel() call


def _get_compiled_nc():
    global _compiled_nc
    if _compiled_nc is None:
        nc = build_nc(S_FULL, S_FULL // 2)
        nc.compile()
        _compiled_nc = nc
    return _compiled_nc


def make_in_maps(x, w):
    """Host-side staging: roll per query half, transpose to d-major,
    duplicate weights along the output dim, pre-scale Wk into the
    Schraudolph integer domain."""
    np_bf16 = mybir.dt.np(BF16)
    qh = S_FULL // 2
    wmod = np.array(w, dtype=np.float32, copy=True)
    wmod[1] *= np.float32(A_K)
    wdup = np.ascontiguousarray(
        np.concatenate([wmod, wmod], axis=2)
    ).astype(np_bf16)  # [3,256,128]
    in_maps = []
    for c in range(N_CORES):
        b, h = c // 2, c % 2
        xb = x[b]
        xr = xb if h == 0 else np.concatenate([xb[qh:], xb[:qh]], axis=0)
        xtc = np.ascontiguousarray(xr.T).reshape(2, P, S_FULL).astype(np_bf16)
        in_maps.append({"xt": xtc, "w": wdup})
    return in_maps


def kernel(x, kernel):
    from concourse.bass_utils import run_bass_kernel_spmd

    x = np.asarray(x, dtype=np.float32)
    w = np.asarray(kernel, dtype=np.float32)
    assert x.shape == (B_FULL, S_FULL, D) and w.shape == (3, D, OUT)
    qh = S_FULL // 2

    nc = _get_compiled_nc()
    res = run_bass_kernel_spmd(nc, make_in_maps(x, w), core_ids=list(range(N_CORES)))
    global LAST_RESULT
    LAST_RESULT = res
    out = np.empty((B_FULL, S_FULL, OUT), dtype=np.float32)
    for c in range(N_CORES):
        b, h = c // 2, c % 2
        num = res.results[c]["num"].astype(np.float64)  # [65, QH]
        out[b, h * qh : (h + 1) * qh] = (num[0:OUT] / num[OUT : OUT + 1]).T
    return out


# revision 56
# speedup vs baseline: 1.0609x; 1.0609x over previous
"""Single-head attention (shared-input QKV projections) on 8 Trainium2 cores.

Reference computation (per batch b):
    q = x[b] @ Wq; k = x[b] @ Wk; v = x[b] @ Wv        # [S, 64]
    out[b] = softmax(q @ k.T / 8) @ v                  # [S, 64]
with B=4, S=4096, D=256, OUT=64.

Sharding: data-parallel over batch (4 batches x 2 cores) with
sequence-parallel query halves; per-core query offset handled by host-side
row rotation of x[b] (attention is permutation-invariant over key rows).

Per-core kernel. The softmax exp stream (8.4M elements/core) would be a
single-engine wall on ScalarE, so whole score tiles alternate between
BOTH ScalarE (true `exp` activation) and VectorE (Schraudolph bit-trick
exp: ONE tensor_scalar add whose int16 output bit pattern IS ~exp in
bf16; ~3% sawtooth error that the softmax normalization largely
cancels -- end-to-end rel err ~6e-3 vs the 2e-2 gate):
  1. DMA x^T (d-major) and W into SBUF, all bf16 (halves HBM traffic;
     the PE rounds fp32 inputs to ~tf32 anyway). Host pre-scales Wk by
     A_K = SCALE*2^7/ln2 so raw scores land in the bf16-Schraudolph
     integer domain; the ACT path undoes this with scale=ln2/2^7.
  2. Projections: Q^T/K^T bf16 (fast-weight-load halves LDWEIGHTS),
     duplicated across both 64-partition halves for 2-way row-packed
     K=64 score matmuls. V is produced directly in [key, out] chunk
     layout by swapping matmul roles (stationary = x chunk, moving =
     Wv), with a ones column per chunk so attn @ V_aug also yields the
     softmax denominator for free.
  3. Scores computed transposed (S^T[k, q]): per (q-block, k-chunk-pair)
     2 row-packed matmuls -> PSUM [128, 2W] (3-deep tile ring, software
     pipelined 2 pairs deep so exp latency never stalls the PE); exp on
     ACT or DVE (by pair parity) -> bf16 SBUF; 2 accumulating
     attn @ V_aug matmuls into PSUM [65, W].
  4. Epilogue: copy PSUM [65, W] numerator+denominator rows to SBUF,
     DMA out. Normalization (divide by the ones-row) and the
     [o,q]->[q,o] transpose happen on the host.
"""

import numpy as np

import concourse.mybir as mybir
import concourse.tile as tile
from concourse import bacc

P = 128
D = 256
OUT = 64
SCALE = 0.125
LN2 = float(np.log(2.0))
A_K = SCALE * (1 << 7) / LN2        # folded into Wk on host (bf16 domain)
ACT_SCALE = LN2 / (1 << 7)          # undoes A_K inside the exp activation
C_MAGIC = 6.0                       # Schraudolph centering (sim-tuned)
B_MAGIC = 127.0 * (1 << 7) - C_MAGIC
F32 = mybir.dt.float32
F32R = mybir.dt.float32r
BF16 = mybir.dt.bfloat16
I16 = mybir.dt.int16

B_FULL, S_FULL = 4, 4096
N_CORES = 8
Z_ACT = 544  # columns of each [128,1024] score tile that ACT exps; DVE rest


def build_nc(S: int, QH: int, QB_W: int = 512, loop_n: int | None = None,
             timing_mode: bool = False, variant: str = "full", bodies: int = 1):
    """Build the per-core SPMD program.

    S: sequence length (key/value rows) held by this core.
    QH: number of query rows this core computes (first QH rows of x).
    QB_W: query block width (free dim of the score matmuls).
    loop_n: if set, run the whole body loop_n times on device (for timing).
    timing_mode: shrink the xt input to 512 cols (replicated on device) so
        host->device transfer noise doesn't swamp loop-delta timing.
    """
    assert S % 512 == 0 and QH % QB_W == 0 and QB_W % P == 0
    nc = bacc.Bacc()
    xt_cols = 512 if timing_mode else S
    xt_in = nc.declare_dram_parameter("xt", [2, P, xt_cols], BF16, isOutput=False)
    w_in = nc.declare_dram_parameter("w", [3, D, P], BF16, isOutput=False)
    num_d = nc.declare_dram_parameter("num", [OUT + 1, QH], F32, isOutput=True)

    with tile.TileContext(nc) as tc:
        with (
            tc.tile_pool(name="const", bufs=1) as constp,
            tc.tile_pool(name="big", bufs=1) as bigp,
            tc.tile_pool(name="attnp", bufs=8) as attnp,
            tc.tile_pool(name="epil", bufs=2) as epilp,
            tc.tile_pool(name="stps", bufs=3, space="PSUM") as stps,
            tc.tile_pool(name="pops", bufs=2, space="PSUM") as pops,
        ):
            miscps = stps  # proj/V scratch shares the score ring pool
            # Weights split across the two HWDGE queues (SP carries q/k,
            # ACT carries v) so the first projection's deps land early.
            w_sb = constp.tile([P, 6 * P], BF16)
            for j in range(3):
                eng = nc.sync if j < 2 else nc.scalar
                for c in range(2):
                    eng.dma_start(
                        w_sb[:, (j * 2 + c) * P : (j * 2 + c + 1) * P],
                        w_in[j, c * P : (c + 1) * P, :],
                    )
            # V_aug lives across loop iterations: the V chunk copies only
            # touch columns 0:64 of each 65-stride group, so the ones
            # column (the softmax-denominator trick) is written once here.
            v_sb = bigp.tile([P, (S // P) * 65], BF16, name="v_sb")
            ones32 = constp.tile([P, S // P], F32)
            nc.vector.memset(ones32, 1.0)
            nc.vector.tensor_copy(
                v_sb.rearrange("p (k c) -> p k c", c=65)[:, :, 64], ones32
            )
            # Trigger the ACT exp table-set load (~2.7us) during the input
            # DMA ramp instead of stalling the first score tile's exp.
            warm = constp.tile([P, 1], BF16, name="warm")
            nc.scalar.activation(
                warm, w_sb[:, 0:1], mybir.ActivationFunctionType.Exp,
                scale=ACT_SCALE,
            )
            if loop_n is not None:
                loop_cm = tc.For_i(0, loop_n, 1)
                loop_cm.__enter__()
            for _ in range(bodies):
                _emit_body(nc, tc, xt_in, num_d, S, QH, QB_W, constp, bigp,
                           attnp, epilp, miscps, stps, pops, w_sb, v_sb,
                           timing_mode, variant)
            if loop_n is not None:
                loop_cm.__exit__(None, None, None)
    return nc


def _emit_body(nc, tc, xt_in, num_d, S, QH, QB_W, constp, bigp, attnp,
               epilp, miscps, stps, pops, w_sb, v_sb, timing_mode=False,
               variant="full"):
    do_dma = variant != "nodma"
    do_st = variant not in ("proj",)
    do_exp = do_st and variant not in ("scores",)
    do_av = do_exp and variant not in ("noav",)
    nk = S // P          # 128-row k chunks
    npair = nk // 2      # row-packed chunk pairs
    nqb = QH // QB_W     # q blocks
    qpb = min(512, QH)   # q-projection block width

    # x^T: chunk c at cols [c*S, (c+1)*S); DMA'd in 1024-col slices so
    # downstream projections wait only on the slices they read. Chunk 0
    # rides the SP HWDGE queue, chunk 1 the ACT queue.
    xt = bigp.tile([P, 2 * S], BF16)
    xb_w = min(512 if timing_mode else 1024, S)
    slices = [(lo, xb_w) for lo in range(0, S, xb_w)]
    if slices[0][1] > 512:
        slices = [(0, 512), (512, xb_w - 512)] + slices[1:]
    if do_dma:
        for lo, wdt in slices:
            for c in range(2):
                eng = nc.sync
                src_lo = 0 if timing_mode else lo
                eng.dma_start(
                    xt[:, c * S + lo : c * S + lo + wdt],
                    xt_in[c, :, src_lo : src_lo + wdt],
                )
    else:
        nc.sync.dma_start(xt[:, 0:512], xt_in[0, :, 0:512])

    qt = bigp.tile([P, QH], BF16)
    kt = bigp.tile([P, S], BF16)

    def proj_block(dst, j, lo, width, eng):
        """dst[:, lo:lo+width] = (W_j^T x^T)[:, lo:lo+width] (d contracted)."""
        pp = miscps.tile([P, width], F32, name="pp", tag="st")
        for c in range(2):
            nc.tensor.matmul(
                pp,
                w_sb[:, (j * 2 + c) * P : (j * 2 + c + 1) * P],
                xt[:, c * S + lo : c * S + lo + width],
                start=(c == 0),
                stop=(c == 1),
            )
        eng(dst[:, lo : lo + width], pp)

    def v_group(g):
        """V rows [g*512,(g+1)*512) directly in [k, o] chunk layout:
        stationary = x chunk (d-major), moving = Wv half."""
        pv = miscps.tile([P, 4 * OUT], F32, name="pv", tag="st")
        for i, kc in enumerate(range(4 * g, 4 * g + 4)):
            for h in range(2):
                nc.tensor.matmul(
                    pv[:, i * OUT : (i + 1) * OUT],
                    xt[:, h * S + kc * P : h * S + (kc + 1) * P],
                    w_sb[:, (4 + h) * P : (4 + h) * P + OUT],
                    start=(h == 0),
                    stop=(h == 1),
                )
        dst = v_sb.rearrange("p (k c) -> p k c", c=65)[:, 4 * g : 4 * g + 4, 0:64]
        nc.vector.tensor_copy(dst, pv.rearrange("p (k c) -> p k c", c=OUT))

    po_tiles = {}

    def st_part(qb, t):
        """Score matmuls (S^T) for q block qb, k chunk pair t -> PSUM tile."""
        qs = qb * QB_W
        kca, kcb = 2 * t, 2 * t + 1
        st = stps.tile([P, 2 * QB_W], F32, name="st", tag="st")
        nc.tensor.matmul(
            st[:, 0:QB_W],
            kt[0:64, kca * P : (kca + 1) * P],
            qt[0:64, qs : qs + QB_W],
            start=True,
            stop=True,
        )
        nc.tensor.matmul(
            st[:, QB_W : 2 * QB_W],
            kt[64:128, kcb * P : (kcb + 1) * P],
            qt[64:128, qs : qs + QB_W],
            start=True,
            stop=True,
        )
        return st

    def exp_part(qb, t, st):
        """exp of a whole score pair-tile on ONE engine, alternating by
        (qb+t) parity: ACT runs the true exp activation, DVE the Schraudolph
        int16 bit-trick. Whole-tile instructions amortize the fixed
        per-instruction engine overhead (ACT: 352 cyc); the 3-deep score
        ring keeps the st-reuse WAR chain off the critical path."""
        at = attnp.tile([P, 2 * QB_W], BF16, name="at", tag="at")
        idx = (qb + t) % 16
        if idx % 2 == 0:
            nc.scalar.activation(
                at, st, mybir.ActivationFunctionType.Exp, scale=ACT_SCALE
            )
        else:
            nc.vector.tensor_scalar(
                at.bitcast(I16), st, B_MAGIC, None, mybir.AluOpType.add
            )
        return at

    def av_part(qb, t, at):
        """attn@V_aug accumulation for q block qb, k chunk pair t."""
        kca, kcb = 2 * t, 2 * t + 1
        po = po_tiles[qb]
        nc.tensor.matmul(
            po,
            v_sb[:, kca * 65 : (kca + 1) * 65],
            at[:, 0:QB_W],
            start=(t == 0),
            stop=False,
        )
        nc.tensor.matmul(
            po,
            v_sb[:, kcb * 65 : (kcb + 1) * 65],
            at[:, QB_W : 2 * QB_W],
            start=False,
            stop=(t == npair - 1),
        )

    def exp_av_part(qb, t, st):
        if not do_exp:
            return
        at = exp_part(qb, t, st)
        if do_av:
            av_part(qb, t, at)

    def epilogue(qb):
        qs = qb * QB_W
        o_sb = epilp.tile([OUT + 1, QB_W], F32, name="o_sb", tag="o_sb")
        if do_av:
            po = po_tiles.pop(qb)
            if qb % 2 == 0:
                nc.scalar.copy(o_sb, po)
            else:
                nc.vector.tensor_copy(o_sb, po)
        else:
            nc.vector.memset(o_sb, 0.0)
        nc.sync.dma_start(num_d[:, qs : qs + QB_W], o_sb)

    # --- emission ---
    # Phase 1: interleave k-chunk production (kt blocks, V groups) with the
    # first two q blocks' score/exp/AV streams, software-pipelined one combo
    # deep so the exp engines never wait on a fresh S^T + semaphore.
    lead = min(2, nqb)
    for lo in range(0, lead * QB_W, qpb):
        proj_block(qt, 0, lo, min(qpb, QH - lo),
                   lambda d, s: nc.vector.tensor_copy(d, s))
    if do_av:
        for qb in range(lead):
            po_tiles[qb] = pops.tile([OUT + 1, QB_W], F32, name="po", tag="po")

    # Software pipeline two score tiles deep (the stps pool holds 3): the
    # exp->attn@V chain of pair p only has to beat TWO later score pairs'
    # PE time, keeping the PE free of exp-latency stalls.
    pend = []

    def push_pend(item):
        pend.append(item)
        if len(pend) > 2:
            exp_av_part(*pend.pop(0))

    def flush_pend():
        while pend:
            exp_av_part(*pend.pop(0))

    for g in range(S // 512):  # 512 k rows per group = 4 chunks = 2 pairs
        proj_block(kt, 1, g * 512, 512,
                   lambda d, s: nc.scalar.copy(d, s))
        v_group(g)
        # Interleave the remaining q projections into phase 1 so the phase
        # boundary has no PE bubble and xt's last reader retires early
        # (lets the next iteration's input DMA start sooner).
        qrest = list(range(lead * QB_W, QH, qpb))
        if g >= 4 and (g - 4) < len(qrest):
            lo = qrest[g - 4]
            proj_block(qt, 0, lo, min(qpb, QH - lo),
                       lambda d, s: nc.vector.tensor_copy(d, s))
        if do_st:
            for t in (2 * g, 2 * g + 1):
                for qb in range(lead):
                    push_pend((qb, t, st_part(qb, t)))
    flush_pend()

    for qb in range(lead):
        epilogue(qb)

    # Phase 2: pure streaming q blocks, software-pipelined one pair deep.
    for qb in range(lead, nqb):
        if do_av:
            po_tiles[qb] = pops.tile([OUT + 1, QB_W], F32, name="po", tag="po")
        if do_st:
            for t in range(npair):
                push_pend((qb, t, st_part(qb, t)))
            flush_pend()
        epilogue(qb)


_compiled_nc = None
LAST_RESULT = None  # BassKernelResults of the most recent kernel() call


def _get_compiled_nc():
    global _compiled_nc
    if _compiled_nc is None:
        nc = build_nc(S_FULL, S_FULL // 2)
        nc.compile()
        _compiled_nc = nc
    return _compiled_nc


def make_in_maps(x, w):
    """Host-side staging: roll per query half, transpose to d-major,
    duplicate weights along the output dim, pre-scale Wk into the
    Schraudolph integer domain."""
    np_bf16 = mybir.dt.np(BF16)
    qh = S_FULL // 2
    wmod = np.array(w, dtype=np.float32, copy=True)
    wmod[1] *= np.float32(A_K)
    wdup = np.ascontiguousarray(
        np.concatenate([wmod, wmod], axis=2)
    ).astype(np_bf16)  # [3,256,128]
    in_maps = []
    for c in range(N_CORES):
        b, h = c // 2, c % 2
        xb = x[b]
        xr = xb if h == 0 else np.concatenate([xb[qh:], xb[:qh]], axis=0)
        xtc = np.ascontiguousarray(xr.T).reshape(2, P, S_FULL).astype(np_bf16)
        in_maps.append({"xt": xtc, "w": wdup})
    return in_maps


def kernel(x, kernel):
    from concourse.bass_utils import run_bass_kernel_spmd

    x = np.asarray(x, dtype=np.float32)
    w = np.asarray(kernel, dtype=np.float32)
    assert x.shape == (B_FULL, S_FULL, D) and w.shape == (3, D, OUT)
    qh = S_FULL // 2

    nc = _get_compiled_nc()
    res = run_bass_kernel_spmd(nc, make_in_maps(x, w), core_ids=list(range(N_CORES)))
    global LAST_RESULT
    LAST_RESULT = res
    out = np.empty((B_FULL, S_FULL, OUT), dtype=np.float32)
    for c in range(N_CORES):
        b, h = c // 2, c % 2
        num = res.results[c]["num"].astype(np.float64)  # [65, QH]
        out[b, h * qh : (h + 1) * qh] = (num[0:OUT] / num[OUT : OUT + 1]).T
    return out
